# revision 10
# baseline (speedup 1.0000x reference)
"""DiffTransformer layer on 8 TRN2 NeuronCores — collective-free shard.

Sharding: core c = (batch b=c//2, parity g=c%2). The 1024 query
positions of a batch are split into 16 blocks of 64; core g owns blocks
j with j%2==g (8 blocks, 512 queries). Causal work is balanced exactly:
both parities see key-tile counts {1..8} over their blocks, so a single
SPMD program serves all cores — only the diagonal-mask constant and the
gathered q-side inputs differ per core. Each core projects q for its
512 positions and k/v for the full sequence (the k/v duplication buys
the removal of both pair ReduceScatters of the previous design), runs
differential attention + per-head subln, out-projection straight into
SBUF, then the full FFN + residual + final RMSNorm for its positions.

Schedule: v(half0) first, then per-pt k/q projections interleaved
between the two attention passes of the previous pt so the ACT-engine
exp (the attention bottleneck) hides under projection matmuls; v(half1)
injected into the first iterations. Scores for key-tile t+1 are emitted
before pv of t to keep independent matmuls ahead of the exp-gated pv
pair in the strict FIFO PE queue. Softmax denominators ride the va
ones-column; division is deferred into the subln rsqrt; partition
broadcasts are selector matmuls with lambda folded host-side. DMAs are
batched into multi-tile transfers (each dma_start costs ~600ns serially
on the sync queue) and pools are scoped tightly so w1 can prefetch
during the attention tail; w2 streams under h2.
"""
import os
import sys
import numpy as np

for _p in ("/opt/trn_rl_repo", "/root/.axon_site/_ro/trn_rl_repo"):
    if os.path.isdir(_p) and _p not in sys.path:
        sys.path.append(_p)

B, S, D, H, HD, FF = 4, 1024, 1024, 16, 32, 4096
NCORES = 8
LAMBDA_INIT = 0.8 - 0.6 * float(np.exp(-0.3 * 12))
EPS = 1e-5
SCALE = float(HD) ** -0.5

SWAP16 = [((i + 16) % 32) for i in range(32)]

LAST_RESULT = None  # BassKernelResults of the most recent run (for test.py)
_PROGRAM = {}


def _build_program():
    import concourse.bacc as bacc
    import concourse.mybir as mybir
    from concourse import tile
    from contextlib import ExitStack

    dt = mybir.dt
    f32, f32r = dt.float32, dt.float32r
    bf16 = dt.bfloat16
    Alu = mybir.AluOpType
    Act = mybir.ActivationFunctionType

    nc = bacc.Bacc("TRN2", target_bir_lowering=False, debug=False,
                   num_devices=NCORES)

    P = 128
    xT = nc.declare_dram_parameter("xT", [D, S], bf16, isOutput=False)
    xQ = nc.declare_dram_parameter("xQ", [D, 512], bf16, isOutput=False)
    wqT = nc.declare_dram_parameter("wqT", [D, D], bf16, isOutput=False)
    wkT = nc.declare_dram_parameter("wkT", [D, D], bf16, isOutput=False)
    wvT = nc.declare_dram_parameter("wvT", [D, D], bf16, isOutput=False)
    woT = nc.declare_dram_parameter("woT", [D, D], bf16, isOutput=False)
    w1s = nc.declare_dram_parameter("w1s", [32, P, 1024], bf16, isOutput=False)
    w2T = nc.declare_dram_parameter("w2T", [FF, D], bf16, isOutput=False)
    b1c = nc.declare_dram_parameter("b1c", [P, 32], f32, isOutput=False)
    b2c = nc.declare_dram_parameter("b2c", [P, 8], f32, isOutput=False)
    rmswc = nc.declare_dram_parameter("rmswc", [P, 8], f32, isOutput=False)
    cosK = nc.declare_dram_parameter("cosK", [P, S], f32, isOutput=False)
    sinK = nc.declare_dram_parameter("sinK", [P, S], f32, isOutput=False)
    cosQ = nc.declare_dram_parameter("cosQ", [P, 512], f32, isOutput=False)
    sinQ = nc.declare_dram_parameter("sinQ", [P, 512], f32, isOutput=False)
    mdiag = nc.declare_dram_parameter("mdiag", [P, P], bf16, isOutput=False)
    hz1l = nc.declare_dram_parameter("hz1l", [P, P], f32r, isOutput=False)
    hz2 = nc.declare_dram_parameter("hz2", [P, P], f32r, isOutput=False)
    hselq = nc.declare_dram_parameter("hselq", [P, P], f32r, isOutput=False)
    hrstd = nc.declare_dram_parameter("hrstd", [16, 1024], f32r,
                                      isOutput=False)
    outT = nc.declare_dram_parameter("outT", [D, 512], f32, isOutput=True)

    def drearr(ap, k=8):
        return ap.rearrange("(k p) c -> p k c", k=k)

    with tile.TileContext(nc) as tc:
        with (
            tc.tile_pool(name="consts", bufs=1) as consts,
        ):
            _atstk = ExitStack()
            atp = _atstk.enter_context(tc.tile_pool(name="attn", bufs=1))
            _kqstk = ExitStack()
            kqp = _kqstk.enter_context(tc.tile_pool(name="kqva", bufs=1))
            _xvstk = ExitStack()
            xvp = _xvstk.enter_context(
                tc.tile_pool(name="xv", bufs=1, side="right"))
            _xqstk = ExitStack()
            xqp = _xqstk.enter_context(
                tc.tile_pool(name="xqw", bufs=1, side="right"))
            _wvstk = ExitStack()
            wvp = _wvstk.enter_context(
                tc.tile_pool(name="wv", bufs=1, side="right"))
            _wqkstk = ExitStack()
            wqkp = _wqkstk.enter_context(
                tc.tile_pool(name="wqk", bufs=1, side="right"))

            xt = xvp.tile([P, 8 * S], bf16, tag="xt")
            xt3 = xt[:].rearrange("p (k s) -> k p s", k=8)
            wv_sb = wvp.tile([P, 8 * 1024], bf16, tag="wv")
            wv3 = wv_sb[:].rearrange("p (k c) -> k p c", k=8)
            xq = xqp.tile([P, 8 * 512], bf16, tag="xq")
            xq3 = xq[:].rearrange("p (k s) -> k p s", k=8)
            wq_sb = wqkp.tile([P, 8 * 1024], bf16, tag="wq")
            wq3 = wq_sb[:].rearrange("p (k c) -> k p c", k=8)
            wk_sb = wqkp.tile([P, 8 * 1024], bf16, tag="wk")
            wk3 = wk_sb[:].rearrange("p (k c) -> k p c", k=8)

            # DMAs ordered by first use: v-proj leads, then k/q
            xtd = xt[:].rearrange("p (k s) -> p k s", k=8)
            nc.sync.dma_start(xtd[:, :, 0:512], drearr(xT[:])[:, :, 0:512])
            nc.sync.dma_start(wv_sb[:].rearrange("p (k c) -> p k c", k=8),
                              drearr(wvT[:]))
            nc.sync.dma_start(xtd[:, :, 512:1024],
                              drearr(xT[:])[:, :, 512:1024])
            nc.sync.dma_start(wk_sb[:].rearrange("p (k c) -> p k c", k=8),
                              drearr(wkT[:]))
            nc.sync.dma_start(wq_sb[:].rearrange("p (k c) -> p k c", k=8),
                              drearr(wqT[:]))
            nc.sync.dma_start(xq[:].rearrange("p (k s) -> p k s", k=8),
                              drearr(xQ[:]))
            cosk_sb = xvp.tile([P, S], f32, tag="cosk")
            sink_sb = xvp.tile([P, S], f32, tag="sink")
            cosq_sb = xqp.tile([P, 512], f32, tag="cosq")
            sinq_sb = xqp.tile([P, 512], f32, tag="sinq")
            nc.sync.dma_start(cosk_sb[:], cosK[:])
            nc.sync.dma_start(sink_sb[:], sinK[:])
            nc.sync.dma_start(cosq_sb[:], cosQ[:])
            nc.sync.dma_start(sinq_sb[:], sinQ[:])

            md_sb = consts.tile([P, P], bf16, tag="md")
            hz1l_sb = consts.tile([P, P], f32r, tag="hz1l")
            hz2_sb = consts.tile([P, P], f32r, tag="hz2")
            hselq_sb = consts.tile([P, P], f32r, tag="hselq")
            hrstd_sb = consts.tile([16, 1024], f32r, tag="hrstd")
            b1_sb = consts.tile([P, 32], f32, tag="b1")
            b2_sb = consts.tile([P, 8], f32, tag="b2")
            rw_sb = consts.tile([P, 8], f32, tag="rw")
            nc.sync.dma_start(md_sb[:], mdiag[:])
            nc.sync.dma_start(hz1l_sb[:], hz1l[:])
            nc.sync.dma_start(hz2_sb[:], hz2[:])
            nc.sync.dma_start(hselq_sb[:], hselq[:])
            nc.sync.dma_start(hrstd_sb[:], hrstd[:])
            nc.sync.dma_start(b1_sb[:], b1c[:])
            nc.sync.dma_start(b2_sb[:], b2c[:])
            nc.sync.dma_start(rw_sb[:], rmswc[:])

            ones_f = consts.tile([P, 16], f32, tag="onesf")
            nc.vector.memset(ones_f[:], 1.0)
            ones_bf = consts.tile([P, 16], bf16, tag="onesb")
            nc.vector.tensor_copy(ones_bf[:], ones_f[:])
            eps_sb = consts.tile([1, 1], f32, tag="eps")
            nc.vector.memset(eps_sb[:], EPS)
            ones_r = consts.tile([P, 1], f32r, tag="onesr")
            nc.vector.tensor_copy(ones_r[:], ones_f[:, 0:1])
            onesw_f = consts.tile([1, P], f32, tag="oneswf")
            nc.vector.memset(onesw_f[:], 1.0)
            h1sel = consts.tile([1, P], f32r, tag="h1sel")
            nc.vector.tensor_copy(h1sel[:], onesw_f[:])

            qT = [kqp.tile([P, 512], bf16, tag=f"qT{i}", name=f"qT{i}")
                  for i in range(8)]
            kT = [kqp.tile([P, S], bf16, tag=f"kT{i}", name=f"kT{i}")
                  for i in range(8)]
            va = [kqp.tile([P, 16 * 65], bf16, tag=f"va{i}", name=f"va{i}")
                  for i in range(8)]
            aTr = [atp.tile([P, 512], bf16, tag=f"ar{i}", name=f"ar{i}")
                   for i in range(8)]

            _w1stk = ExitStack()
            _wostk = ExitStack()
            with (
                tc.tile_pool(name="st_ps", bufs=2, space="PSUM") as stp,
                tc.tile_pool(name="pv_ps", bufs=2, space="PSUM") as pvp,
                tc.tile_pool(name="ms_ps", bufs=1, space="PSUM") as msp,
                tc.tile_pool(name="bc_ps", bufs=1, space="PSUM") as bcp,
                tc.tile_pool(name="rtmp", bufs=2) as rtmp,
                tc.tile_pool(name="epool", bufs=2) as epool,
                tc.tile_pool(name="apool", bufs=1) as apool,
                tc.tile_pool(name="zpool", bufs=1) as zpool,
                tc.tile_pool(name="post", bufs=1) as post,
            ):
                aw = [(apool.tile([P, 512], bf16, tag=f"a1_{pt}",
                                  name=f"a1_{pt}"),
                       apool.tile([P, 512], bf16, tag=f"a2_{pt}",
                                  name=f"a2_{pt}")) for pt in range(8)]
                zpt = [zpool.tile([P, 512], f32r, tag=f"z{pt}",
                                  name=f"z{pt}") for pt in range(8)]
                poswq = {}
                for pt in range(8):
                    nc.vector.memset(zpt[pt][:].bitcast(f32), 0.0)
                ms = msp.tile([16, 512], f32, tag="ms", name="ms")

                def project_v(st, half):
                    ps = pvp.tile([P, 512], f32, tag="pv", name="vps")
                    for kd in range(8):
                        nc.tensor.matmul(
                            ps[:],
                            lhsT=xt3[kd, :, st * P:(st + 1) * P],
                            rhs=wv3[kd, :, half * 512:(half + 1) * 512],
                            start=(kd == 0), stop=(kd == 7))
                    va3 = va[st][:].rearrange("p (h e) -> p h e", h=16, e=65)
                    nc.vector.tensor_copy(
                        va3[:, 8 * half:8 * half + 8, 0:64],
                        ps[:].rearrange("p (h e) -> p h e", h=8, e=64))
                    nc.vector.tensor_copy(
                        va3[:, 8 * half:8 * half + 8, 64:65],
                        ones_bf[:, 8 * half:8 * half + 8]
                        .rearrange("p (h o) -> p h o", o=1))

                def rope(dst, ps, cos_sb, sin_sb, c0, n):
                    tmp = rtmp.tile([P, 512], f32, tag="rt", name="rt")
                    nc.vector.stream_shuffle(tmp[:, 0:n], ps, SWAP16)
                    nc.vector.tensor_tensor(dst, ps, cos_sb[:, c0:c0 + n],
                                            Alu.mult)
                    tmp2 = rtmp.tile([P, 512], bf16, tag="rt2", name="rt2")
                    nc.vector.tensor_tensor(tmp2[:, 0:n], tmp[:, 0:n],
                                            sin_sb[:, c0:c0 + n], Alu.mult)
                    nc.gpsimd.tensor_tensor(dst, dst, tmp2[:, 0:n], Alu.add)

                def project_k(mt):
                    ps = stp.tile([P, 1024], f32, tag="st", name="kps")
                    ps3 = ps[:].rearrange("p (c n) -> p c n", c=2, n=512)
                    for kd in range(8):
                        lh = wk3[kd, :, mt * P:(mt + 1) * P]
                        nc.tensor.matmul(ps3[:, 0, :], lhsT=lh,
                                         rhs=xt3[kd, :, 0:512],
                                         start=(kd == 0), stop=(kd == 7))
                        nc.tensor.matmul(ps3[:, 1, :], lhsT=lh,
                                         rhs=xt3[kd, :, 512:1024],
                                         start=(kd == 0), stop=(kd == 7))
                    for ch in range(2):
                        rope(kT[mt][:, ch * 512:(ch + 1) * 512],
                             ps3[:, ch, :], cosk_sb, sink_sb, ch * 512, 512)

                def project_q(mt):
                    ps = stp.tile([P, 1024], f32, tag="st", name="qps")
                    for kd in range(8):
                        nc.tensor.matmul(
                            ps[:, 0:512],
                            lhsT=wq3[kd, :, mt * P:(mt + 1) * P],
                            rhs=xq3[kd, :, :],
                            start=(kd == 0), stop=(kd == 7))
                    rope(qT[mt][:], ps[:, 0:512], cosq_sb, sinq_sb, 0, 512)

                md3 = md_sb[:].rearrange("p (g n) -> p g n", g=2)

                def attn_pass(pt, h):
                    # head hv=2pt+h, branch pair rows (2h, 2h+1)*32 of tile
                    pvA = pvp.tile([65, 512], f32, tag="pv", name="pvA")
                    pvB = pvp.tile([65, 512], f32, tag="pv", name="pvB")
                    hv = 2 * pt + h
                    es = {}

                    def emit_scores(t):
                        j0 = 64 * t
                        st = stp.tile([P, 1024], f32, tag="st", name="st")
                        st3 = st[:].rearrange("p (g n) -> p g n", g=2, n=512)
                        for gi, g in enumerate((2 * h, 2 * h + 1)):
                            nc.tensor.matmul(
                                st3[:, gi, j0:],
                                lhsT=kT[pt][g * 32:(g + 1) * 32,
                                            t * P:(t + 1) * P],
                                rhs=qT[pt][g * 32:(g + 1) * 32, j0:],
                                start=True, stop=True,
                                tile_position=(g * 32, 0))
                        e = epool.tile([P, 1024], bf16, tag="e", name="e")
                        e3 = e[:].rearrange("p (g n) -> p g n", g=2, n=512)
                        nc.scalar.activation(e3[:, :, j0:], st3[:, :, j0:],
                                             Act.Exp, scale=SCALE)
                        nc.vector.tensor_tensor(
                            e3[:, :, j0:j0 + 64], e3[:, :, j0:j0 + 64],
                            md3, Alu.mult)
                        es[t] = e3

                    def emit_pv(t):
                        j0 = 64 * t
                        e3 = es.pop(t)
                        nc.tensor.matmul(
                            pvA[:, j0:], lhsT=va[t][:, hv * 65:hv * 65 + 65],
                            rhs=e3[:, 0, j0:],
                            start=(t == 0), stop=(t == 7))
                        nc.tensor.matmul(
                            pvB[:, j0:], lhsT=va[t][:, hv * 65:hv * 65 + 65],
                            rhs=e3[:, 1, j0:],
                            start=(t == 0), stop=(t == 7))

                    emit_scores(0)
                    for t in range(8):
                        if t + 1 < 8:
                            emit_scores(t + 1)
                        emit_pv(t)
                    last = (pt == 7 and h == 1)
                    a1, a2 = aw[pt]
                    nc.vector.tensor_copy(a1[64 * h:64 * h + 64, :],
                                          pvA[0:64, :])
                    (nc.scalar.copy if last else nc.vector.tensor_copy)(
                        a2[64 * h:64 * h + 64, :], pvB[0:64, :])
                    z = zpt[pt]
                    (nc.scalar.copy if last else nc.vector.tensor_copy)(
                        z[32 * h:32 * h + 1, :], pvA[64:65, :])
                    nc.vector.tensor_copy(z[64 + 32 * h:65 + 32 * h, :],
                                          pvB[64:65, :])

                def emit_posw(pt):
                    # posw = Z2*A1 - lam*Z1*A2 ; sq = posw^2/8 ; ms rows
                    a1, a2 = aw[pt]
                    z = zpt[pt]
                    bc2 = bcp.tile([P, 512], f32, tag="bc", name="bc2")
                    nc.tensor.matmul(bc2[:], lhsT=hz2_sb[:],
                                     rhs=z[:], start=True, stop=True)
                    t1 = post.tile([P, 512], f32r, tag="t1")
                    nc.vector.tensor_tensor(t1[:], a1[:], bc2[:], Alu.mult)
                    bc1 = bcp.tile([P, 512], f32, tag="bc", name="bc1")
                    nc.tensor.matmul(bc1[:], lhsT=hz1l_sb[:],
                                     rhs=z[:], start=True, stop=True)
                    t2 = post.tile([P, 512], f32r, tag="t2")
                    nc.vector.tensor_tensor(t2[:], a2[:], bc1[:], Alu.mult)
                    posw = post.tile([P, 512], bf16, tag="posw", bufs=8,
                                     name="posw")
                    nc.vector.tensor_tensor(posw[:], t1[:], t2[:],
                                            Alu.subtract)
                    sq = post.tile([P, 512], f32r, tag="t1", name="sq")
                    nc.vector.scalar_tensor_tensor(
                        sq[:], in0=posw[:], scalar=0.125, in1=posw[:],
                        op0=Alu.mult, op1=Alu.mult)
                    nc.tensor.matmul(ms[:],
                                     lhsT=hselq_sb[:, 16 * pt:16 * pt + 16],
                                     rhs=sq[:], start=(pt == 0),
                                     stop=(pt == 7))
                    poswq[pt] = posw

                def emit_rstd():
                    srt = rtmp.tile([16, 512], f32, tag="rt", name="srt")
                    nc.scalar.activation(srt[:], ms[:], Act.Sqrt,
                                         scale=1.0 / (1.0 - LAMBDA_INIT) ** 2)
                    rstd = rtmp.tile([16, 512], f32, tag="rt",
                                     name="rstd")
                    nc.vector.reciprocal_approx_fast(rstd[:], srt[:])
                    rstr = post.tile([16, 512], f32r, tag="t2", name="rstr")
                    nc.vector.tensor_copy(rstr[:], rstd[:])
                    return rstr[:]

                def emit_apply(pt, rstd):
                    pool = pvp if pt % 2 else bcp
                    tg = "pv" if pt % 2 else "bc"
                    bcr = pool.tile([P, 512], f32, tag=tg, name="bcr")
                    nc.tensor.matmul(bcr[:],
                                     lhsT=hrstd_sb[:, pt * P:(pt + 1) * P],
                                     rhs=rstd, start=True, stop=True)
                    nc.vector.tensor_tensor(attnT[pt][:], poswq[pt][:],
                                            bcr[:], Alu.mult)

                # ---- schedule -----------------------------------------
                for st in range(8):
                    project_v(st, 0)
                project_k(0)
                project_q(0)
                vh1 = {0: (0, 1, 2), 1: (3, 4, 5), 2: (6, 7)}
                kq = {0: (1, 2), 1: (3, 4, 5), 2: (6, 7)}
                for pt in range(8):
                    if pt == 3:
                        _wqkstk.close()
                        _wvstk.close()
                        _xqstk.close()
                        _xvstk.close()
                        wop = _wostk.enter_context(
                            tc.tile_pool(name="wo", bufs=1, side="right"))
                        wo_sb = wop.tile([P, 8 * 1024], bf16, tag="wo")
                        wo3 = wo_sb[:].rearrange("p (k c) -> k p c", k=8)
                        nc.sync.dma_start(
                            wo_sb[:].rearrange("p (k c) -> p k c", k=8),
                            drearr(woT[:]))
                        w1p = _w1stk.enter_context(
                            tc.tile_pool(name="w1p", bufs=1, side="right"))
                        w1_sb = w1p.tile([P, 32 * 1024], bf16, tag="w1")
                        w13 = w1_sb[:].rearrange("p (m c) -> m p c", m=32)
                        w1d = w1_sb[:].rearrange("p (m c) -> p m c", m=32)
                        for grp in range(4):
                            nc.sync.dma_start(
                                w1d[:, 8 * grp:8 * grp + 8, :],
                                w1s[8 * grp:8 * grp + 8, :, :]
                                .rearrange("m p c -> p m c"))
                    for st in vh1.get(pt, ()):
                        project_v(st, 1)
                    attn_pass(pt, 0)
                    for mt in kq.get(pt, ()):
                        project_k(mt)
                        project_q(mt)
                    attn_pass(pt, 1)
                    if pt > 0:
                        emit_posw(pt - 1)
                emit_posw(7)
                rstd = emit_rstd()
                attnT = [kqp.tile([P, 512], bf16, tag=f"qT{i}",
                                  name=f"at{i}") for i in range(8)]
                for pt in range(8):
                    emit_apply(pt, rstd)

                # ---- out-projection straight into SBUF ----------------
                for mo in range(8):
                    ps = stp.tile([P, 1024], f32, tag="st", name="wops")
                    for kc in range(8):
                        nc.tensor.matmul(
                            ps[:, 0:512],
                            lhsT=wo3[kc, :, mo * P:(mo + 1) * P],
                            rhs=attnT[kc][:],
                            start=(kc == 0), stop=(kc == 7))
                    nc.vector.tensor_copy(aTr[mo][:], ps[:, 0:512])

            _kqstk.close()

            # ---- FFN + residual + final RMS -------------------------
            with (
                tc.tile_pool(name="h1", bufs=1) as h1p,
                tc.tile_pool(name="w2p", bufs=4) as w2p,
                tc.tile_pool(name="yT", bufs=1) as ytp,
                tc.tile_pool(name="fin", bufs=2) as finp,
                tc.tile_pool(name="sm2", bufs=1) as sm2,
            ):
                h1 = [h1p.tile([P, 512], bf16, tag=f"h1_{i}", name=f"h1_{i}")
                      for i in range(32)]
                with tc.tile_pool(name="h1_ps", bufs=4, space="PSUM") as h1ps:
                    for mf in range(32):
                        ps = h1ps.tile([P, 512], f32, tag="h1ps",
                                       name="h1ps")
                        for kd in range(8):
                            nc.tensor.matmul(
                                ps[:], lhsT=w13[mf, :, kd * P:(kd + 1) * P],
                                rhs=aTr[kd][:], start=(kd == 0),
                                stop=(kd == 7))
                        nc.scalar.activation(h1[mf][:], ps[:], Act.Relu,
                                             bias=b1_sb[:, mf:mf + 1])
                _w1stk.close()
                _wostk.close()

                # h2: 8 persistent PSUM accumulators, stream w2 tiles
                yt = [ytp.tile([P, 512], f32, tag=f"y{i}", name=f"y{i}")
                      for i in range(8)]
                with tc.tile_pool(name="h2_ps", bufs=1, space="PSUM") as h2ps:
                    ps8 = [h2ps.tile([P, 512], f32, tag=f"h2_{mo}",
                                     name=f"h2_{mo}") for mo in range(8)]
                    for kf in range(32):
                        wt2 = w2p.tile([P, 1024], bf16, tag="w2t",
                                       name="w2t")
                        nc.sync.dma_start(wt2[:], w2T[kf * P:(kf + 1) * P, :])
                        for mo in range(8):
                            nc.tensor.matmul(
                                ps8[mo][:], lhsT=wt2[:, mo * P:(mo + 1) * P],
                                rhs=h1[kf][:], start=(kf == 0),
                                stop=(kf == 31))
                    for mo in range(8):
                        nc.vector.scalar_tensor_tensor(
                            yt[mo][:], in0=ps8[mo][:],
                            scalar=b2_sb[:, mo:mo + 1], in1=aTr[mo][:],
                            op0=Alu.add, op1=Alu.add)

                with tc.tile_pool(name="rms_ps", bufs=1,
                                  space="PSUM") as rmsps:
                    ms_ps = rmsps.tile([P, 512], f32, tag="rmsps",
                                       name="rmsps")
                    for mo in range(8):
                        sq = finp.tile([P, 512], f32r, tag="fsq", name="fsq")
                        nc.scalar.activation(sq[:], yt[mo][:], Act.Square)
                        nc.tensor.matmul(ms_ps[0:1, :], lhsT=ones_r[:],
                                         rhs=sq[:], start=(mo == 0),
                                         stop=(mo == 7))
                    srt = sm2.tile([1, 512], f32, tag="fsrt")
                    nc.scalar.activation(srt[:], ms_ps[0:1, :], Act.Sqrt,
                                         scale=1.0 / 1024.0, bias=eps_sb[:])
                    rstd2 = sm2.tile([1, 512], f32, tag="frstd")
                    nc.vector.reciprocal_approx_fast(rstd2[:], srt[:])
                    rstr2 = sm2.tile([1, 512], f32r, tag="frstr")
                    nc.vector.tensor_copy(rstr2[:], rstd2[:])
                    bcr = rmsps.tile([P, 512], f32, tag="fbc", name="fbc")
                    nc.tensor.matmul(bcr[:], lhsT=h1sel[:], rhs=rstr2[:],
                                     start=True, stop=True)
                    for mo in range(8):
                        ot = finp.tile([P, 512], f32, tag="fot", name="fot")
                        nc.vector.scalar_tensor_tensor(
                            ot[:], in0=yt[mo][:], scalar=rw_sb[:, mo:mo + 1],
                            in1=bcr[:], op0=Alu.mult, op1=Alu.mult)
                        nc.sync.dma_start(outT[mo * P:(mo + 1) * P, :], ot[:])

            _atstk.close()

    nc.compile()
    return nc


def _qcols(g):
    # core-local column c -> global seq position
    return np.concatenate(
        [np.arange(128 * i + 64 * g, 128 * i + 64 * g + 64)
         for i in range(8)])


def _host_prep(inputs):
    import ml_dtypes
    bfloat16 = ml_dtypes.bfloat16
    x = np.asarray(inputs["x"], dtype=np.float32)
    Wq = np.asarray(inputs["Wq"], dtype=np.float32)
    Wk = np.asarray(inputs["Wk"], dtype=np.float32)
    Wv = np.asarray(inputs["Wv"], dtype=np.float32)
    Wo = np.asarray(inputs["Wo"], dtype=np.float32)
    W1 = np.asarray(inputs["W1"], dtype=np.float32)
    b1 = np.asarray(inputs["b1"], dtype=np.float32)
    W2 = np.asarray(inputs["W2"], dtype=np.float32)
    b2 = np.asarray(inputs["b2"], dtype=np.float32)
    rmsw = np.asarray(inputs["rms_weight"], dtype=np.float32)
    lam = float(np.exp(np.dot(np.asarray(inputs["lambda_q1"], np.float64),
                              np.asarray(inputs["lambda_k1"], np.float64)))
                - np.exp(np.dot(np.asarray(inputs["lambda_q2"], np.float64),
                                np.asarray(inputs["lambda_k2"], np.float64)))
                + LAMBDA_INIT)

    half = HD // 2
    freqs = (1.0 / (10000.0 ** (np.arange(half, dtype=np.float32)
                                / np.float32(half)))).astype(np.float32)
    ang = (np.arange(S, dtype=np.float32)[:, None] * freqs[None, :])
    cos16 = np.cos(ang.astype(np.float32)).T.astype(np.float32)
    sin16 = np.sin(ang.astype(np.float32)).T.astype(np.float32)
    cosK_full = np.ascontiguousarray(
        np.tile(np.concatenate([cos16, cos16], 0), (4, 1)))
    sinK_full = np.ascontiguousarray(
        np.tile(np.concatenate([-sin16, sin16], 0), (4, 1)))
    perm32 = np.concatenate([np.arange(0, 32, 2), np.arange(1, 32, 2)])
    permed = np.concatenate([c0 * 32 + perm32 for c0 in range(32)])

    wqT_h = np.ascontiguousarray(Wq[permed, :].T.astype(bfloat16))
    wkT_h = np.ascontiguousarray(Wk[permed, :].T.astype(bfloat16))
    wvT_h = np.ascontiguousarray(Wv.T.astype(bfloat16))
    woT_h = np.ascontiguousarray(Wo.T.astype(bfloat16))
    w1s = np.ascontiguousarray(
        W1.T.reshape(8, 128, 32, 128).transpose(2, 1, 0, 3)
        .reshape(32, 128, 1024).astype(bfloat16))
    w2T_h = np.ascontiguousarray(W2.T.astype(bfloat16))
    b1c = np.ascontiguousarray(b1.reshape(32, 128).T)
    b2c = np.ascontiguousarray(b2.reshape(8, 128).T)
    rmswc = np.ascontiguousarray(rmsw.reshape(8, 128).T)

    # selector constants (role-independent)
    hz1l = np.zeros((128, 128), np.float32)
    hz2 = np.zeros((128, 128), np.float32)
    for h in range(2):
        pcols = slice(64 * h, 64 * h + 64)
        hz1l[32 * h, pcols] = lam
        hz2[64 + 32 * h, pcols] = 1.0
    hselq = np.zeros((128, 128), np.float32)
    hrstd = np.zeros((16, 1024), np.float32)
    for pt in range(8):
        for h in range(2):
            rows = slice(64 * h, 64 * h + 64)
            hselq[rows, 16 * pt + 2 * pt + h] = 0.125
            hrstd[2 * pt + h, 128 * pt + 64 * h:128 * pt + 64 * h + 64] = 1.0

    # per-parity diagonal masks (keys r=0..127 down, q offset o=0..63)
    r = np.arange(128)[:, None]
    o = np.arange(64)[None, :]
    md_g = []
    for g in range(2):
        if g == 0:
            m = (r < 64) & (r <= o)
        else:
            m = (r < 64) | ((r - 64) <= o)
        md_g.append(np.ascontiguousarray(
            np.tile(m.astype(np.float32), (1, 2)).astype(bfloat16)))

    in_maps = []
    for c in range(NCORES):
        b, g = c // 2, c % 2
        qc = _qcols(g)
        xTb = x[b].T.astype(bfloat16)
        in_maps.append({
            "xT": np.ascontiguousarray(xTb),
            "xQ": np.ascontiguousarray(xTb[:, qc]),
            "wqT": wqT_h, "wkT": wkT_h, "wvT": wvT_h, "woT": woT_h,
            "w1s": w1s, "w2T": w2T_h,
            "b1c": b1c, "b2c": b2c, "rmswc": rmswc,
            "cosK": cosK_full, "sinK": sinK_full,
            "cosQ": np.ascontiguousarray(cosK_full[:, qc]),
            "sinQ": np.ascontiguousarray(sinK_full[:, qc]),
            "mdiag": md_g[g],
            "hz1l": hz1l, "hz2": hz2, "hselq": hselq, "hrstd": hrstd,
        })
    return in_maps


def kernel(**inputs):
    global LAST_RESULT
    from concourse.bass_utils import run_bass_kernel_spmd

    if "nc" not in _PROGRAM:
        _PROGRAM["nc"] = _build_program()
    nc = _PROGRAM["nc"]

    in_maps = _host_prep(inputs)
    trace = bool(int(os.environ.get("KERNEL_TRACE", "0")))
    res = run_bass_kernel_spmd(nc, in_maps, list(range(NCORES)), trace=trace)
    LAST_RESULT = res

    out = np.empty((B, S, D), np.float32)
    for c in range(NCORES):
        b, g = c // 2, c % 2
        out[b, _qcols(g), :] = res.results[c]["outT"].T
    return out
